# revision 10
# baseline (speedup 1.0000x reference)
"""Trainium2 Bass kernel for the MAB dense-transformer block.

Sharding (zero cross-core communication): 8 cores; core c handles batch
b = c//2 and query-row half c%2 (1024 of 2048 rows), all 8 heads.
Each core receives pre-transposed (feature-major) inputs so every matmul
operand is contraction-major in SBUF, and returns its output feature-major;
the host transposes back and assembles the full (4, 2048, 1024) output.
"""

import math
import sys

import numpy as np

sys.path.insert(0, "/opt/trn_rl_repo")

P = 128
D = 1024          # model dim
NI = 1024         # query rows per core
NK = 2048         # key rows
H = 8             # heads
FT = D // P       # feature tiles
KTI = D // P      # contraction tiles
JT = NK // P      # key tiles
NCH = 512         # matmul free-dim chunk (one PSUM bank of fp32)
ICH = NI // NCH   # 2
SCALE = 1.0 / math.sqrt(128.0)
EPS = 1e-5

_built = None


def _build():
    global _built
    if _built is not None:
        return _built

    from contextlib import ExitStack
    from concourse import bacc, tile, mybir

    f32 = mybir.dt.float32
    bf16 = mybir.dt.bfloat16
    f32r = mybir.dt.float32r
    FX = mybir.ActivationFunctionType

    def r(ap):
        return ap

    nc = bacc.Bacc(None, target_bir_lowering=False, debug=False)

    QT_d = nc.dram_tensor("QT", [D, NI], bf16, kind="ExternalInput")
    KT_d = nc.dram_tensor("KT", [D, NK], bf16, kind="ExternalInput")
    WqT_d = nc.dram_tensor("WqT", [D, D], bf16, kind="ExternalInput")
    WkT_d = nc.dram_tensor("WkT", [D, D], bf16, kind="ExternalInput")
    WvT_d = nc.dram_tensor("WvT", [D, D], bf16, kind="ExternalInput")
    WoT_d = nc.dram_tensor("WoT", [D, D], bf16, kind="ExternalInput")
    vrow_d = nc.dram_tensor("vrow", [8, D], f32, kind="ExternalInput")
    vpt_d = nc.dram_tensor("vpt", [P, 8, 8], f32, kind="ExternalInput")
    OT_d = nc.dram_tensor("OT", [D, NI], f32, kind="ExternalOutput")

    QT_r = QT_d[:].rearrange("(t p) i -> p t i", p=P)
    KT_r = KT_d[:].rearrange("(t p) j -> p t j", p=P)
    WqT_r = WqT_d[:].rearrange("(t p) f -> p t f", p=P)
    WkT_r = WkT_d[:].rearrange("(t p) f -> p t f", p=P)
    WvT_r = WvT_d[:].rearrange("(t p) f -> p t f", p=P)
    WoT_r = WoT_d[:].rearrange("(t p) f -> p t f", p=P)
    OT_r = OT_d[:].rearrange("(t p) i -> p t i", p=P)

    # vrow rows: 0 bq, 1 bk, 2 bv, 3 bo, 4 g0, 5 b0, 6 g1, 7 b1
    # vpt[p, v, t] = vec_v[t*128 + p]

    with tile.TileContext(nc) as tc:
        with ExitStack() as stack:
            constp = stack.enter_context(tc.tile_pool(name="constp", bufs=1))
            oTp = stack.enter_context(tc.tile_pool(name="oTp", bufs=1))
            projstack = ExitStack()
            projout = projstack.enter_context(tc.tile_pool(name="projout", bufs=1))
            qTb = projout.tile([P, H, NI], bf16)
            kTb = projout.tile([P, H, NK], bf16)
            vb = projout.tile([P, JT, D], bf16)
            ones_row = constp.tile([1, NCH], f32)
            ones_colb = constp.tile([P, 1], bf16)
            ones_colf = constp.tile([P, 1], f32)
            eps_t = constp.tile([1, 1], f32)
            vp_sb = constp.tile([P, 8, 8], f32)
            bv_t = constp.tile([1, D], f32)
            nc.vector.memset(ones_row, 1.0)
            nc.vector.memset(ones_colb, 1.0)
            nc.vector.memset(ones_colf, 1.0)
            nc.vector.memset(eps_t, EPS)
            nc.sync.dma_start(out=vp_sb, in_=vpt_d[:])
            nc.sync.dma_start(out=bv_t, in_=vrow_d[2:3, :])

            # ---------------- P1: q projection (out feature-major) ----------
            with tc.tile_pool(name="p1", bufs=1) as p1, \
                 tc.tile_pool(name="p1ps", bufs=4, space="PSUM") as p1ps:
                QT_sb = p1.tile([P, KTI, NI], bf16)
                Wq_sb = p1.tile([P, KTI, D], bf16)
                nc.sync.dma_start(out=QT_sb, in_=QT_r)
                nc.sync.dma_start(out=Wq_sb, in_=WqT_r)
                for ft in range(FT):
                    for ic in range(ICH):
                        ps = p1ps.tile([P, NCH], f32)
                        for kt in range(KTI):
                            nc.tensor.matmul(
                                ps,
                                r(Wq_sb[:, kt, ft * P:(ft + 1) * P]),
                                r(QT_sb[:, kt, ic * NCH:(ic + 1) * NCH]),
                                start=(kt == 0), stop=(kt == KTI - 1))
                        sl = slice(ic * NCH, (ic + 1) * NCH)
                        # + bq (varies along partition here)
                        nc.vector.tensor_scalar_add(
                            out=qTb[:, ft, sl], in0=ps, scalar1=vp_sb[:, 0, ft:ft + 1])

            # ---------------- P2a: k projection (feature-major) -------------
            with tc.tile_pool(name="p2a", bufs=1) as p2a, \
                 tc.tile_pool(name="p2aps", bufs=4, space="PSUM") as p2aps:
                Wk_sb = p2a.tile([P, KTI, D], bf16)
                nc.sync.dma_start(out=Wk_sb, in_=WkT_r)
                for jh in range(2):
                    KT_sb = p2a.tile([P, KTI, NK // 2], bf16, tag="kthalf")
                    nc.sync.dma_start(
                        out=KT_sb, in_=KT_r[:, :, jh * (NK // 2):(jh + 1) * (NK // 2)])
                    for ft in range(FT):
                        for jc in range(2):
                            ps = p2aps.tile([P, NCH], f32)
                            for kt in range(KTI):
                                nc.tensor.matmul(
                                    ps,
                                    r(Wk_sb[:, kt, ft * P:(ft + 1) * P]),
                                    r(KT_sb[:, kt, jc * NCH:(jc + 1) * NCH]),
                                    start=(kt == 0), stop=(kt == KTI - 1))
                            sl = slice(jh * 1024 + jc * NCH, jh * 1024 + (jc + 1) * NCH)
                            nc.vector.tensor_scalar_add(
                                out=kTb[:, ft, sl], in0=ps, scalar1=vp_sb[:, 1, ft:ft + 1])

            # ---------------- P2b: v projection (key-row-major) -------------
            with tc.tile_pool(name="p2b", bufs=1) as p2b, \
                 tc.tile_pool(name="p2bps", bufs=4, space="PSUM") as p2bps:
                Wv_sb = p2b.tile([P, KTI, D], bf16)
                nc.sync.dma_start(out=Wv_sb, in_=WvT_r)
                for jh in range(2):
                    KT_sb = p2b.tile([P, KTI, NK // 2], bf16, tag="kthalf2")
                    nc.sync.dma_start(
                        out=KT_sb, in_=KT_r[:, :, jh * (NK // 2):(jh + 1) * (NK // 2)])
                    for jl in range(8):
                        jt = jh * 8 + jl
                        for ec in range(ICH):
                            ps = p2bps.tile([P, NCH], f32)
                            for kt in range(KTI):
                                nc.tensor.matmul(
                                    ps,
                                    r(KT_sb[:, kt, jl * P:(jl + 1) * P]),
                                    r(Wv_sb[:, kt, ec * NCH:(ec + 1) * NCH]),
                                    start=(kt == 0), stop=False)
                            # + bv (varies along free dim): rank-1 ones x bv
                            nc.tensor.matmul(
                                ps,
                                r(ones_row[0:1, 0:P]),
                                r(bv_t[0:1, ec * NCH:(ec + 1) * NCH]),
                                start=False, stop=True)
                            nc.vector.tensor_copy(
                                out=vb[:, jt, ec * NCH:(ec + 1) * NCH], in_=ps)

            # ---------------- P3: attention ---------------------------------
            with tc.tile_pool(name="att", bufs=3) as att, \
                 tc.tile_pool(name="atts", bufs=2) as atts, \
                 tc.tile_pool(name="psS", bufs=2, space="PSUM") as psS, \
                 tc.tile_pool(name="psO", bufs=2, space="PSUM") as psO, \
                 tc.tile_pool(name="psC", bufs=2, space="PSUM") as psC:
                oT = oTp.tile([P, FT, NI], f32)
                for h in range(H):
                    o_ps = [psO.tile([P, NCH], f32, name=f"ops{_i}", tag="ops") for _i in range(ICH)]
                    c_ps = [psC.tile([1, NCH], f32, name=f"cps{_i}", tag="cps") for _i in range(ICH)]
                    for jt in range(JT):
                        s_ps = psS.tile([P, NI], f32, tag="sps")
                        for ic in range(ICH):
                            nc.tensor.matmul(
                                s_ps[:, ic * NCH:(ic + 1) * NCH],
                                kTb[:, h, jt * P:(jt + 1) * P],
                                qTb[:, h, ic * NCH:(ic + 1) * NCH],
                                start=True, stop=True)
                        e_t = att.tile([P, NI], bf16, tag="expt")
                        nc.scalar.activation(out=e_t, in_=s_ps, func=FX.Exp,
                                             scale=SCALE)
                        for ic in range(ICH):
                            esl = e_t[:, ic * NCH:(ic + 1) * NCH]
                            nc.tensor.matmul(
                                o_ps[ic], vb[:, jt, h * P:(h + 1) * P], esl,
                                start=(jt == 0), stop=(jt == JT - 1))
                            nc.tensor.matmul(
                                c_ps[ic], ones_colb, esl,
                                start=(jt == 0), stop=(jt == JT - 1))
                    # normalize + residual
                    cs_sb = atts.tile([1, NI], f32, tag="cssb")
                    rec_t = atts.tile([1, NI], f32, tag="rect")
                    for ic in range(ICH):
                        nc.vector.tensor_copy(
                            out=cs_sb[:, ic * NCH:(ic + 1) * NCH], in_=c_ps[ic])
                    nc.vector.reciprocal(out=rec_t, in_=cs_sb)
                    rb_ps = psS.tile([P, NI], f32, tag="sps")
                    for ic in range(ICH):
                        nc.tensor.matmul(
                            rb_ps[:, ic * NCH:(ic + 1) * NCH],
                            r(ones_row[0:1, 0:P]),
                            r(rec_t[0:1, ic * NCH:(ic + 1) * NCH]),
                            start=True, stop=True)
                    o_sb = att.tile([P, NI], f32, tag="osb")
                    for ic in range(ICH):
                        nc.vector.tensor_copy(
                            out=o_sb[:, ic * NCH:(ic + 1) * NCH], in_=o_ps[ic])
                    nc.vector.tensor_mul(out=o_sb, in0=o_sb, in1=rb_ps)
                    nc.vector.tensor_add(out=oT[:, h, :], in0=o_sb, in1=qTb[:, h, :])

            # ------------- P4: LayerNorm 1 (feature axis = partitions) ------
            projstack.close()
            lnp = stack.enter_context(tc.tile_pool(name="lnp", bufs=1))
            ln1 = lnp.tile([P, FT, NI], f32)
            _layernorm_fmajor(nc, tc, oT, ln1, vp_sb, 4, 5,
                              ones_colf, eps_t, ones_row, f32, f32r, FX)
            ln1b = lnp.tile([P, FT, NI], bf16)
            for ft in range(FT):
                nc.vector.tensor_copy(out=ln1b[:, ft, :], in_=ln1[:, ft, :])

            # ---------------- P5: MLP + residual ----------------------------
            with tc.tile_pool(name="p5", bufs=1) as p5:
                mstack = ExitStack()
                p5t = mstack.enter_context(tc.tile_pool(name="p5t", bufs=3))
                p5ps = mstack.enter_context(
                    tc.tile_pool(name="p5ps", bufs=4, space="PSUM"))
                Wo_sb = p5.tile([P, KTI, D], bf16)
                nc.sync.dma_start(out=Wo_sb, in_=WoT_r)
                oT2 = p5.tile([P, FT, NI], f32)
                for et in range(FT):
                    for ic in range(ICH):
                        ps = p5ps.tile([P, NCH], f32)
                        for kt in range(KTI):
                            nc.tensor.matmul(
                                ps,
                                Wo_sb[:, kt, et * P:(et + 1) * P],
                                ln1b[:, kt, ic * NCH:(ic + 1) * NCH],
                                start=(kt == 0), stop=(kt == KTI - 1))
                        z_sb = p5t.tile([P, NCH], f32, tag="zsb")
                        # relu(Z + bo): bo varies along partition (e)
                        nc.scalar.activation(out=z_sb, in_=ps, func=FX.Relu,
                                             bias=vp_sb[:, 3, et:et + 1])
                        sl = slice(ic * NCH, (ic + 1) * NCH)
                        nc.vector.tensor_add(
                            out=oT2[:, et, sl], in0=ln1[:, et, sl], in1=z_sb)

                # ------------- P6: LayerNorm 2 + store ----------------------
                mstack.close()
                out_sb = p5.tile([P, FT, NI], f32)
                _layernorm_fmajor(nc, tc, oT2, out_sb, vp_sb, 6, 7,
                                  ones_colf, eps_t, ones_row, f32, f32r, FX)
                nc.sync.dma_start(out=OT_r, in_=out_sb)

    nc.compile()
    _built = nc
    return nc


def _layernorm_fmajor(nc, tc, x, out, vp_sb, gi, bi, ones_colf, eps_t,
                      ones_row, f32, f32r, FX):
    """LayerNorm over the partition (feature) axis of x [P, FT, NI]."""
    from concourse import mybir

    def r(ap):
        return ap

    with tc.tile_pool(name="lnt", bufs=2) as lnt, \
         tc.tile_pool(name="lnr", bufs=1) as lnr, \
         tc.tile_pool(name="lnps", bufs=1, space="PSUM") as lnps, \
         tc.tile_pool(name="lnbc", bufs=1, space="PSUM") as lnbc:
        sum_ps = lnps.tile([1, NI], f32, tag="lsum")
        sq_ps = lnps.tile([1, NI], f32, tag="lsq")
        for ft in range(FT):
            sq_t = lnt.tile([P, NI], f32, tag="sqt")
            nc.scalar.activation(out=sq_t, in_=x[:, ft, :], func=FX.Square)
            for ic in range(ICH):
                sl = slice(ic * NCH, (ic + 1) * NCH)
                nc.tensor.matmul(sum_ps[:, sl], r(ones_colf), r(x[:, ft, sl]),
                                 start=(ft == 0), stop=(ft == FT - 1))
                nc.tensor.matmul(sq_ps[:, sl], r(ones_colf), r(sq_t[:, sl]),
                                 start=(ft == 0), stop=(ft == FT - 1))
        mu = lnr.tile([1, NI], f32)
        ex2 = lnr.tile([1, NI], f32)
        var = lnr.tile([1, NI], f32)
        rstd = lnr.tile([1, NI], f32)
        nmr = lnr.tile([1, NI], f32)
        nc.scalar.mul(out=mu, in_=sum_ps, mul=1.0 / D)
        nc.scalar.mul(out=ex2, in_=sq_ps, mul=1.0 / D)
        nc.vector.tensor_mul(out=var, in0=mu, in1=mu)
        nc.vector.tensor_sub(out=var, in0=ex2, in1=var)
        nc.scalar.activation(out=var, in_=var, func=FX.Sqrt, bias=eps_t)
        nc.vector.reciprocal(out=rstd, in_=var)
        nc.vector.tensor_mul(out=nmr, in0=mu, in1=rstd)
        nc.scalar.mul(out=nmr, in_=nmr, mul=-1.0)
        # broadcast rstd and (-mu*rstd) across partitions via rank-1 matmul
        a_ps = lnbc.tile([P, NI], f32, tag="abc")
        b_ps = lnbc.tile([P, NI], f32, tag="bbc")
        for ic in range(ICH):
            sl = slice(ic * NCH, (ic + 1) * NCH)
            nc.tensor.matmul(a_ps[:, sl], r(ones_row[0:1, 0:P]),
                             r(rstd[0:1, sl]), start=True, stop=True)
            nc.tensor.matmul(b_ps[:, sl], r(ones_row[0:1, 0:P]),
                             r(nmr[0:1, sl]), start=True, stop=True)
        for ft in range(FT):
            nc.vector.tensor_mul(out=out[:, ft, :], in0=x[:, ft, :], in1=a_ps)
            nc.vector.tensor_add(out=out[:, ft, :], in0=out[:, ft, :], in1=b_ps)
            nc.vector.tensor_scalar(
                out=out[:, ft, :], in0=out[:, ft, :],
                scalar1=vp_sb[:, gi, ft:ft + 1], scalar2=vp_sb[:, bi, ft:ft + 1],
                op0=mybir.AluOpType.mult, op1=mybir.AluOpType.add)


def _in_maps(inputs):
    import ml_dtypes
    bf = ml_dtypes.bfloat16
    Q = np.asarray(inputs["Q"], np.float32)
    K = np.asarray(inputs["K"], np.float32)
    WqT = np.ascontiguousarray(np.asarray(inputs["Wq"], np.float32).T).astype(bf)
    WkT = np.ascontiguousarray(np.asarray(inputs["Wk"], np.float32).T).astype(bf)
    WvT = np.ascontiguousarray(np.asarray(inputs["Wv"], np.float32).T).astype(bf)
    WoT = np.ascontiguousarray(np.asarray(inputs["Wo"], np.float32).T).astype(bf)
    vrow = np.stack([np.asarray(inputs[k], np.float32) for k in
                     ("bq", "bk", "bv", "bo", "g0", "b0", "g1", "b1")])
    vpt = np.ascontiguousarray(vrow.reshape(8, 8, 128).transpose(2, 0, 1))
    maps = []
    for c in range(8):
        b, half = c // 2, c % 2
        maps.append({
            "QT": np.ascontiguousarray(Q[b, half * NI:(half + 1) * NI, :].T).astype(bf),
            "KT": np.ascontiguousarray(K[b].T).astype(bf),
            "WqT": WqT, "WkT": WkT, "WvT": WvT, "WoT": WoT,
            "vrow": vrow, "vpt": vpt,
        })
    return maps


def _run(in_maps, trace=False):
    from concourse.bass_utils import run_bass_kernel_spmd
    nc = _build()
    return run_bass_kernel_spmd(nc, in_maps, list(range(8)), trace=trace)


def kernel(**inputs):
    res = _run(_in_maps(inputs))
    out = np.empty((4, 2048, 1024), np.float32)
    for c in range(8):
        b, half = c // 2, c % 2
        out[b, half * NI:(half + 1) * NI, :] = res.results[c]["OT"].T
    return out
